# revision 4
# baseline (speedup 1.0000x reference)
"""Q8 linear layer (dequant matmul) on 8 Trainium2 NeuronCores.

out[t, o] = sum_i (x[t, i] * scales[i]) * weight[o, i]

Sharding: tensor-parallel over out_features (14336 = 8 * 1792). Each core
gets the pre-scaled activations (bf16) and a 1792-column slice of weight^T.
int8-valued weights are exact in bf16.

V2 pipeline (per core):
  - weight shipped as INT8 (1 B/elem HBM -- half the traffic of bf16) in a
    host-prearranged partition-contiguous layout [128, OB, KT, OBS], so each
    DMA chunk is 128 descriptors x 7168 B contiguous.
  - on-chip upcast int8->bf16, split across DVE / ACT / GPSIMD by k-tiles
    (SPLIT), overlapped with the DMA stream.
  - col-tiled matmuls: 2 k-tiles run concurrently in 2 PE column groups
    (tile_position), accumulated over 16 rounds per output block; folded
    64->32 partitions with one ACT psum->sbuf copy + one DVE add (the BIR
    verifier forbids TensorTensor reading two PSUM operands).
  - output blocks complete sequentially, so fold + out-DMA of block i
    overlaps the DMA/upcast/matmul of block i+1.
"""

import os
import sys

for _p in ("/opt/trn_rl_repo", "/root/.axon_site/_ro/trn_rl_repo"):
    if os.path.isdir(_p) and _p not in sys.path:
        sys.path.insert(0, _p)

import numpy as np
import ml_dtypes

import concourse.bass as bass
import concourse.mybir as mybir
import concourse.tile as tile
from concourse import bacc
from concourse.bass_utils import run_bass_kernel_spmd

TOKENS = 32
IN_F = 4096
OUT_F = 14336
NCORES = 8
OPC = OUT_F // NCORES  # 1792 out features per core
KT = IN_F // 128  # 32 k-tiles
OB = 4  # output column blocks per core
OBS = OPC // OB  # 448 columns per block (one PSUM bank)
HK = 16  # k-tiles per weight DMA chunk
NH = KT // HK  # chunks per output block

# int8->bf16 upcast split (k-tiles per chunk): DVE, ACT, GPSIMD
SPLIT = (7, 5, 4)

_cached_nc = {}


def _emit_body(nc, pools, aps):
    xpool, w8pool, wbfpool, fpool, opool, pspool = pools
    xs_r, w8_r, out_r = aps

    xs_sb = xpool.tile([128, KT, TOKENS], mybir.dt.bfloat16, name="xs_sb", tag="xs")
    nc.sync.dma_start(out=xs_sb[:], in_=xs_r)

    d, a, g = SPLIT
    assert d + a + g == HK

    for ob in range(OB):
        ps = pspool.tile([64, OBS], mybir.dt.float32, name=f"ps_{ob}", tag="ps")
        for h in range(NH):
            w8 = w8pool.tile(
                [128, HK, OBS], mybir.dt.int8, name=f"w8_{ob}_{h}", tag="w8"
            )
            nc.sync.dma_start(out=w8[:], in_=w8_r[ob][h])
            wbf = wbfpool.tile(
                [128, HK, OBS], mybir.dt.bfloat16, name=f"wbf_{ob}_{h}", tag="wbf"
            )
            nc.vector.tensor_copy(wbf[:, 0:d, :], w8[:, 0:d, :])
            nc.scalar.copy(wbf[:, d : d + a, :], w8[:, d : d + a, :])
            if g:
                nc.gpsimd.tensor_copy(wbf[:, d + a : HK, :], w8[:, d + a : HK, :])
            for rr in range(HK // 2):
                for j in range(2):
                    kk = 2 * rr + j
                    ki = h * HK + kk
                    nc.tensor.matmul(
                        ps[32 * j : 32 * (j + 1), :],
                        xs_sb[:, ki, :],
                        wbf[:, kk, :],
                        start=(h == 0 and rr == 0),
                        stop=(h == NH - 1 and rr == HK // 2 - 1),
                        tile_position=(0, 32 * j),
                        # sim's zero-region group check drops the partition
                        # base of col-group strips; disjoint strips are safe
                        skip_group_check=True,
                    )
        # fold partitions 32:64 onto 0:32 (one PSUM read per instruction)
        s1 = fpool.tile([TOKENS, OBS], mybir.dt.float32, name=f"s1_{ob}", tag="s1")
        nc.scalar.copy(s1[:, :], ps[32:64, :])
        o_sb = opool.tile([TOKENS, OBS], mybir.dt.float32, name=f"o_{ob}", tag="o")
        nc.vector.tensor_add(o_sb[:, :], ps[0:32, :], s1[:, :])
        nc.sync.dma_start(out=out_r[ob], in_=o_sb[:])


def _build():
    key = ("v2", HK, SPLIT)
    if key in _cached_nc:
        return _cached_nc[key]

    nc = bacc.Bacc(
        "TRN2", target_bir_lowering=False, debug=False, num_devices=NCORES
    )
    xs = nc.dram_tensor(
        "xs", [128, KT, TOKENS], mybir.dt.bfloat16, kind="ExternalInput"
    )
    w8 = nc.dram_tensor(
        "w8", [128, OB, KT, OBS], mybir.dt.int8, kind="ExternalInput"
    )
    out = nc.dram_tensor(
        "out", [TOKENS, OPC], mybir.dt.float32, kind="ExternalOutput"
    )

    xs_r = xs.ap()
    w8_r = w8.ap().rearrange("p ob (h k) c -> ob h p k c", h=NH)
    out_r = out.ap().rearrange("t (ob c) -> ob t c", ob=OB)
    aps = (xs_r, w8_r, out_r)

    with tile.TileContext(nc) as tc:
        with (
            tc.tile_pool(name="xpool", bufs=1) as xpool,
            tc.tile_pool(name="w8pool", bufs=3) as w8pool,
            tc.tile_pool(name="wbfpool", bufs=3) as wbfpool,
            tc.tile_pool(name="fpool", bufs=2) as fpool,
            tc.tile_pool(name="opool", bufs=2) as opool,
            tc.tile_pool(name="pspool", bufs=2, space=bass.MemorySpace.PSUM) as pspool,
        ):
            pools = (xpool, w8pool, wbfpool, fpool, opool, pspool)
            _emit_body(nc, pools, aps)

    nc.compile()
    _cached_nc[key] = nc
    return nc


def make_in_maps(x, weight, scales):
    x = np.asarray(x, dtype=np.float32)
    weight = np.asarray(weight)
    scales = np.asarray(scales, dtype=np.float32)
    assert x.shape == (TOKENS, IN_F) and weight.shape == (OUT_F, IN_F)

    xs = (x * scales[None, :]).astype(ml_dtypes.bfloat16)
    # [p, k, t]: xs_dram[p, k, t] = xs[t, k*128 + p]
    xs_dram = np.ascontiguousarray(xs.T.reshape(KT, 128, TOKENS).transpose(1, 0, 2))

    w8f = weight.astype(np.int8)
    in_maps = []
    for c in range(NCORES):
        wc = w8f[c * OPC : (c + 1) * OPC, :]  # [1792, 4096]
        # w8[p, ob, k, cc] = wc[ob*OBS + cc, k*128 + p]
        w8c = np.ascontiguousarray(
            wc.reshape(OB, OBS, KT, 128).transpose(3, 0, 2, 1)
        )
        in_maps.append({"xs": xs_dram, "w8": w8c})
    return in_maps


def run(x, weight, scales, trace=False, trace_cores=None):
    nc = _build()
    in_maps = make_in_maps(x, weight, scales)
    res = run_bass_kernel_spmd(
        nc,
        in_maps,
        core_ids=list(range(NCORES)),
        trace=trace,
        trace_cores=trace_cores,
    )
    out = np.concatenate(
        [res.results[c]["out"] for c in range(NCORES)], axis=1
    ).astype(np.float32, copy=False)
    return out, res


def kernel(x, weight, scales):
    out, _ = run(x, weight, scales)
    return out


# revision 5
# speedup vs baseline: 1.7489x; 1.7489x over previous
"""Q8 linear layer (dequant matmul) on 8 Trainium2 NeuronCores.

out[t, o] = sum_i (x[t, i] * scales[i]) * weight[o, i]

Sharding: tensor-parallel over out_features (14336 = 8 * 1792). Each core
gets the pre-scaled activations (bf16) and a 1792-column slice of weight^T.
int8-valued weights are exact in bf16.

V3 pipeline (per core), balanced at ~21us/engine:
  - weight k-tiles 0..13 of each 16-k chunk shipped as INT8 (1 B/elem HBM),
    host-prearranged partition-contiguous; upcast on-chip int8->bf16:
    9 k-tiles on DVE (CAST hits the 2x two-port mode, ~224 Gelem/s) and
    5 on ACT (~130 Gelem/s). GPSIMD must stay idle: its ops and DVE
    two-port ops fight over an exclusive shared SBUF port pair.
  - k-tiles 14..15 shipped directly as bf16 DMA into the same SBUF tile,
    soaking the HBM-read headroom the engines leave.
  - col-tiled matmuls: 2 k-tiles run concurrently in 2 PE column groups
    (tile_position), accumulated over 16 rounds per output block; folded
    64->32 partitions with one ACT psum->sbuf copy + one DVE add (the BIR
    verifier forbids TensorTensor reading two PSUM operands).
  - output blocks complete sequentially, so fold + out-DMA of block i
    overlaps the DMA/upcast/matmul of block i+1.
"""

import os
import sys

for _p in ("/opt/trn_rl_repo", "/root/.axon_site/_ro/trn_rl_repo"):
    if os.path.isdir(_p) and _p not in sys.path:
        sys.path.insert(0, _p)

import numpy as np
import ml_dtypes

import concourse.bass as bass
import concourse.mybir as mybir
import concourse.tile as tile
from concourse import bacc
from concourse.bass_utils import run_bass_kernel_spmd

TOKENS = 32
IN_F = 4096
OUT_F = 14336
NCORES = 8
OPC = OUT_F // NCORES  # 1792 out features per core
KT = IN_F // 128  # 32 k-tiles
OB = 4  # output column blocks per core
OBS = OPC // OB  # 448 columns per block (one PSUM bank)
HK = 16  # k-tiles per weight DMA chunk
NH = KT // HK  # chunks per output block

# per-chunk k-tile split: DVE-cast, ACT-cast, direct-bf16-DMA
ND, NA, NB = 9, 5, 2
N8 = ND + NA  # int8 k-tiles per chunk

_cached_nc = {}


def _emit_body(nc, pools, aps):
    xpool, w8pool, wbfpool, fpool, opool, pspool = pools
    xs_r, w8_r, w16_r, out_r = aps

    xs_sb = xpool.tile([128, KT, TOKENS], mybir.dt.bfloat16, name="xs_sb", tag="xs")
    nc.sync.dma_start(out=xs_sb[:], in_=xs_r)

    for ob in range(OB):
        ps = pspool.tile([64, OBS], mybir.dt.float32, name=f"ps_{ob}", tag="ps")
        for h in range(NH):
            w8 = w8pool.tile(
                [128, N8, OBS], mybir.dt.int8, name=f"w8_{ob}_{h}", tag="w8"
            )
            nc.sync.dma_start(out=w8[:], in_=w8_r[ob][h])
            wbf = wbfpool.tile(
                [128, HK, OBS], mybir.dt.bfloat16, name=f"wbf_{ob}_{h}", tag="wbf"
            )
            nc.sync.dma_start(out=wbf[:, N8:HK, :], in_=w16_r[ob][h])
            nc.vector.tensor_copy(wbf[:, 0:ND, :], w8[:, 0:ND, :])
            nc.scalar.copy(wbf[:, ND:N8, :], w8[:, ND:N8, :])
            for rr in range(HK // 2):
                for j in range(2):
                    kk = 2 * rr + j
                    ki = h * HK + kk
                    nc.tensor.matmul(
                        ps[32 * j : 32 * (j + 1), :],
                        xs_sb[:, ki, :],
                        wbf[:, kk, :],
                        start=(h == 0 and rr == 0),
                        stop=(h == NH - 1 and rr == HK // 2 - 1),
                        tile_position=(0, 32 * j),
                        # sim's zero-region group check drops the partition
                        # base of col-group strips; disjoint strips are safe
                        skip_group_check=True,
                    )
        # fold partitions 32:64 onto 0:32 (one PSUM read per instruction)
        s1 = fpool.tile([TOKENS, OBS], mybir.dt.float32, name=f"s1_{ob}", tag="s1")
        nc.scalar.copy(s1[:, :], ps[32:64, :])
        o_sb = opool.tile([TOKENS, OBS], mybir.dt.float32, name=f"o_{ob}", tag="o")
        nc.vector.tensor_add(o_sb[:, :], ps[0:32, :], s1[:, :])
        nc.sync.dma_start(out=out_r[ob], in_=o_sb[:])


def _build():
    key = ("v3", HK, ND, NA, NB)
    if key in _cached_nc:
        return _cached_nc[key]

    nc = bacc.Bacc(
        "TRN2", target_bir_lowering=False, debug=False, num_devices=NCORES
    )
    xs = nc.dram_tensor(
        "xs", [128, KT, TOKENS], mybir.dt.bfloat16, kind="ExternalInput"
    )
    w8 = nc.dram_tensor(
        "w8", [128, OB, NH, N8, OBS], mybir.dt.int8, kind="ExternalInput"
    )
    w16 = nc.dram_tensor(
        "w16", [128, OB, NH, NB, OBS], mybir.dt.bfloat16, kind="ExternalInput"
    )
    out = nc.dram_tensor(
        "out", [TOKENS, OPC], mybir.dt.float32, kind="ExternalOutput"
    )

    xs_r = xs.ap()
    w8_r = w8.ap().rearrange("p ob h k c -> ob h p k c")
    w16_r = w16.ap().rearrange("p ob h k c -> ob h p k c")
    out_r = out.ap().rearrange("t (ob c) -> ob t c", ob=OB)
    aps = (xs_r, w8_r, w16_r, out_r)

    with tile.TileContext(nc) as tc:
        with (
            tc.tile_pool(name="xpool", bufs=1) as xpool,
            tc.tile_pool(name="w8pool", bufs=3) as w8pool,
            tc.tile_pool(name="wbfpool", bufs=3) as wbfpool,
            tc.tile_pool(name="fpool", bufs=2) as fpool,
            tc.tile_pool(name="opool", bufs=2) as opool,
            tc.tile_pool(name="pspool", bufs=2, space=bass.MemorySpace.PSUM) as pspool,
        ):
            pools = (xpool, w8pool, wbfpool, fpool, opool, pspool)
            _emit_body(nc, pools, aps)

    nc.compile()
    _cached_nc[key] = nc
    return nc


def make_in_maps(x, weight, scales):
    x = np.asarray(x, dtype=np.float32)
    weight = np.asarray(weight)
    scales = np.asarray(scales, dtype=np.float32)
    assert x.shape == (TOKENS, IN_F) and weight.shape == (OUT_F, IN_F)

    xs = (x * scales[None, :]).astype(ml_dtypes.bfloat16)
    # [p, k, t]: xs_dram[p, k, t] = xs[t, k*128 + p]
    xs_dram = np.ascontiguousarray(xs.T.reshape(KT, 128, TOKENS).transpose(1, 0, 2))

    w8f = weight.astype(np.int8)
    in_maps = []
    for c in range(NCORES):
        wc = w8f[c * OPC : (c + 1) * OPC, :]  # [1792, 4096]
        # [p, ob, ki, cc] with ki = h*HK + kk
        wt = np.ascontiguousarray(
            wc.reshape(OB, OBS, KT, 128).transpose(3, 0, 2, 1)
        )  # [128, OB, KT, OBS]
        wt = wt.reshape(128, OB, NH, HK, OBS)
        w8c = np.ascontiguousarray(wt[:, :, :, 0:N8, :])
        w16c = np.ascontiguousarray(wt[:, :, :, N8:HK, :]).astype(ml_dtypes.bfloat16)
        in_maps.append({"xs": xs_dram, "w8": w8c, "w16": w16c})
    return in_maps


def run(x, weight, scales, trace=False, trace_cores=None):
    nc = _build()
    in_maps = make_in_maps(x, weight, scales)
    res = run_bass_kernel_spmd(
        nc,
        in_maps,
        core_ids=list(range(NCORES)),
        trace=trace,
        trace_cores=trace_cores,
    )
    out = np.concatenate(
        [res.results[c]["out"] for c in range(NCORES)], axis=1
    ).astype(np.float32, copy=False)
    return out, res


def kernel(x, weight, scales):
    out, _ = run(x, weight, scales)
    return out


# revision 6
# speedup vs baseline: 2.0008x; 1.1440x over previous
"""Q8 linear layer (dequant matmul) on 8 Trainium2 NeuronCores.

out[t, o] = sum_i (x[t, i] * scales[i]) * weight[o, i]

Sharding: tensor-parallel over out_features (14336 = 8 * 1792). Each core
gets the pre-scaled activations (bf16) and a 1792-column slice of weight^T.
int8-valued weights are exact in bf16.

V4 pipeline (per core):
  - weight split per chunk into (DVE-cast int8, ACT-cast int8, direct bf16)
    k-tiles. DVE CAST runs the 2x two-port mode (~227 Gelem/s measured);
    ACT COPY ~130 Gelem/s; both overlap the DMA stream. GPSIMD stays idle
    (its ops and DVE two-port ops fight over an exclusive SBUF port pair).
  - all weight-chunk DMAs issue upfront on the in-order sync queue into
    uniquely-tagged SBUF tiles (no pool recycling -> no issue stalls).
  - first chunks are small so the engines start early; the tail chunks are
    direct bf16 so the last matmuls chase the DMA with no cast dependency.
  - col-tiled matmuls: 2 k-tiles concurrently in 2 PE column groups
    (tile_position); fold 64->32 partitions = ACT psum->sbuf copy + DVE add
    (BIR forbids TensorTensor reading two PSUM operands), emitted one output
    block late so DVE casts never queue behind fold waits.
"""

import os
import sys

for _p in ("/opt/trn_rl_repo", "/root/.axon_site/_ro/trn_rl_repo"):
    if os.path.isdir(_p) and _p not in sys.path:
        sys.path.insert(0, _p)

import numpy as np
import ml_dtypes

import concourse.bass as bass
import concourse.mybir as mybir
import concourse.tile as tile
from concourse import bacc
from concourse.bass_utils import run_bass_kernel_spmd

TOKENS = 32
IN_F = 4096
OUT_F = 14336
NCORES = 8
OPC = OUT_F // NCORES  # 1792 out features per core
KT = IN_F // 128  # 32 k-tiles
OB = 4  # output column blocks per core
OBS = OPC // OB  # 448 columns per block (one PSUM bank)

# per output block: list of chunks (n_dve_int8, n_act_int8, n_bf16) k-tiles
CHUNKS = [
    [(5, 3, 0), (5, 3, 0), (11, 5, 0)],
    [(10, 6, 0), (11, 5, 0)],
    [(10, 6, 0), (11, 5, 0)],
    [(2, 4, 10), (0, 0, 16)],
]
assert all(sum(c) % 2 == 0 for cl in CHUNKS for c in cl)
assert all(sum(sum(c) for c in cl) == KT for cl in CHUNKS)

_cached_nc = {}


def _chunk_list():
    out = []
    for ob, cl in enumerate(CHUNKS):
        koff = 0
        for i, (nd, na, nb) in enumerate(cl):
            out.append((ob, i, koff, nd, na, nb))
            koff += nd + na + nb
    return out


def _emit_body(nc, pools, aps):
    cpool, fpool, opool, pspool = pools
    xs_r, w8_rs, w16_rs, out_r = aps
    chunks = _chunk_list()

    xs_sb = cpool.tile([128, KT, TOKENS], mybir.dt.bfloat16, name="xs_sb", tag="xs")
    nc.sync.dma_start(out=xs_sb[:], in_=xs_r)

    # all weight DMAs upfront, in chunk order
    w8_sb, wbf_sb = {}, {}
    for ob, i, koff, nd, na, nb in chunks:
        nk = nd + na + nb
        wbf = cpool.tile(
            [128, nk, OBS], mybir.dt.bfloat16, name=f"wbf_{ob}_{i}", tag=f"wbf{ob}{i}"
        )
        wbf_sb[(ob, i)] = wbf
        if nd + na:
            w8 = cpool.tile(
                [128, nd + na, OBS], mybir.dt.int8,
                name=f"w8_{ob}_{i}", tag=f"w8{ob}{i}",
            )
            w8_sb[(ob, i)] = w8
            nc.sync.dma_start(out=w8[:], in_=w8_rs[(ob, i)])
        if nb:
            nc.sync.dma_start(out=wbf[:, nd + na : nk, :], in_=w16_rs[(ob, i)])

    ps = {}
    pending_fold = []

    def emit_fold(ob):
        p = ps[ob]
        s1 = fpool.tile([TOKENS, OBS], mybir.dt.float32, name=f"s1_{ob}", tag="s1")
        nc.scalar.copy(s1[:, :], p[32:64, :])
        o_sb = opool.tile([TOKENS, OBS], mybir.dt.float32, name=f"o_{ob}", tag="o")
        nc.vector.tensor_add(o_sb[:, :], p[0:32, :], s1[:, :])
        nc.sync.dma_start(out=out_r[ob], in_=o_sb[:])

    for ob, cl in enumerate(CHUNKS):
        ps[ob] = pspool.tile([64, OBS], mybir.dt.float32, name=f"ps_{ob}", tag="ps")
        nrounds = KT // 2
        r = 0
        for i, (nd, na, nb) in enumerate(cl):
            nk = nd + na + nb
            koff = sum(sum(c) for c in cl[:i])
            wbf = wbf_sb[(ob, i)]
            if nd:
                nc.vector.tensor_copy(wbf[:, 0:nd, :], w8_sb[(ob, i)][:, 0:nd, :])
            if na:
                nc.scalar.copy(
                    wbf[:, nd : nd + na, :], w8_sb[(ob, i)][:, nd : nd + na, :]
                )
            # previous block's fold goes in after this block's first casts so
            # its psum/ACT deps are already met when DVE/ACT reach it
            if i == 0 and pending_fold:
                emit_fold(pending_fold.pop())
            for rr in range(nk // 2):
                for j in range(2):
                    kk = 2 * rr + j
                    ki = koff + kk
                    nc.tensor.matmul(
                        ps[ob][32 * j : 32 * (j + 1), :],
                        xs_sb[:, ki, :],
                        wbf[:, kk, :],
                        start=(r == 0),
                        stop=(r == nrounds - 1),
                        tile_position=(0, 32 * j),
                        # sim's zero-region group check drops the partition
                        # base of col-group strips; disjoint strips are safe
                        skip_group_check=True,
                    )
                r += 1
        pending_fold.append(ob)
    emit_fold(pending_fold.pop())


def _build():
    key = ("v4", str(CHUNKS))
    if key in _cached_nc:
        return _cached_nc[key]

    nc = bacc.Bacc(
        "TRN2", target_bir_lowering=False, debug=False, num_devices=NCORES
    )
    xs = nc.dram_tensor(
        "xs", [128, KT, TOKENS], mybir.dt.bfloat16, kind="ExternalInput"
    )
    w8_rs, w16_rs = {}, {}
    for ob, i, koff, nd, na, nb in _chunk_list():
        if nd + na:
            t = nc.dram_tensor(
                f"w8_{ob}_{i}", [128, nd + na, OBS], mybir.dt.int8,
                kind="ExternalInput",
            )
            w8_rs[(ob, i)] = t.ap()
        if nb:
            t = nc.dram_tensor(
                f"w16_{ob}_{i}", [128, nb, OBS], mybir.dt.bfloat16,
                kind="ExternalInput",
            )
            w16_rs[(ob, i)] = t.ap()
    out = nc.dram_tensor(
        "out", [TOKENS, OPC], mybir.dt.float32, kind="ExternalOutput"
    )

    xs_r = xs.ap()
    out_r = out.ap().rearrange("t (ob c) -> ob t c", ob=OB)
    aps = (xs_r, w8_rs, w16_rs, out_r)

    with tile.TileContext(nc) as tc:
        with (
            tc.tile_pool(name="cpool", bufs=1) as cpool,
            tc.tile_pool(name="fpool", bufs=2) as fpool,
            tc.tile_pool(name="opool", bufs=2) as opool,
            tc.tile_pool(name="pspool", bufs=4, space=bass.MemorySpace.PSUM) as pspool,
        ):
            pools = (cpool, fpool, opool, pspool)
            _emit_body(nc, pools, aps)

    nc.compile()
    _cached_nc[key] = nc
    return nc


def make_in_maps(x, weight, scales):
    x = np.asarray(x, dtype=np.float32)
    weight = np.asarray(weight)
    scales = np.asarray(scales, dtype=np.float32)
    assert x.shape == (TOKENS, IN_F) and weight.shape == (OUT_F, IN_F)

    xs = (x * scales[None, :]).astype(ml_dtypes.bfloat16)
    # [p, k, t]: xs_dram[p, k, t] = xs[t, k*128 + p]
    xs_dram = np.ascontiguousarray(xs.T.reshape(KT, 128, TOKENS).transpose(1, 0, 2))

    w8f = weight.astype(np.int8)
    chunks = _chunk_list()
    in_maps = []
    for c in range(NCORES):
        wc = w8f[c * OPC : (c + 1) * OPC, :]  # [1792, 4096]
        # [p, ob, ki, cc]
        wt = np.ascontiguousarray(
            wc.reshape(OB, OBS, KT, 128).transpose(3, 0, 2, 1)
        )
        m = {"xs": xs_dram}
        for ob, i, koff, nd, na, nb in chunks:
            blk = wt[:, ob, koff : koff + nd + na + nb, :]
            if nd + na:
                m[f"w8_{ob}_{i}"] = np.ascontiguousarray(blk[:, 0 : nd + na, :])
            if nb:
                m[f"w16_{ob}_{i}"] = np.ascontiguousarray(
                    blk[:, nd + na :, :]
                ).astype(ml_dtypes.bfloat16)
        in_maps.append(m)
    return in_maps


def run(x, weight, scales, trace=False, trace_cores=None):
    nc = _build()
    in_maps = make_in_maps(x, weight, scales)
    res = run_bass_kernel_spmd(
        nc,
        in_maps,
        core_ids=list(range(NCORES)),
        trace=trace,
        trace_cores=trace_cores,
    )
    out = np.concatenate(
        [res.results[c]["out"] for c in range(NCORES)], axis=1
    ).astype(np.float32, copy=False)
    return out, res


def kernel(x, weight, scales):
    out, _ = run(x, weight, scales)
    return out
